# revision 32
# baseline (speedup 1.0000x reference)
"""DiscriminativeLoss kernel for 8 trn2 NeuronCores (Bass/Tile), v2.

Sharding: core c handles image b = c//2, pixel half h = c%2 (NPIX = 524288
pixels per core).  Host casts the embedding shard to bf16 in TWO layouts
(pixel-major for pass 1, kj-interleaved for pass 2) so each is DMA'd once
with long contiguous lines.

Device, per core:
  pass 1: per-class counts + segment sums over the pixel shard via 4096
          one-hot matmuls [128px, 9cls] x [128px, 16emb + ones-col]
          accumulating into one PSUM tile [9, 17] (col 16 = counts).
          Classes 1..9 only -- class 0 (background) is excluded from the
          loss so its stats are never needed.
  AllReduce of the [9, 17] stats over 2-core pair groups (cores sharing
          an image), giving per-image stats on both cores.
  pass 2: per-pixel hinge terms.  7-pixel-group chunks (f2=512):
          PSUM[(9k,7j), f] = -2 e.C_k + (s_n + 2*BIG*kk*lab - BIG*lab^2)
          via 2 matmuls (s1: centers stationary; s23: emb^2+lab+lab^2
          moving rows with constant stationary).  Six chunks share a
          [128, 1536] PSUM supergroup (upper half via tile_position
          (0,64)), then one Relu ACT (u) + one Sqrt ACT (y), each with
          per-partition accumulation.  Wrong-class/background lanes give
          u=0, y=delta, which the +2*delta^2*Npp correction cancels
          exactly; host zero-pads the 1024-pixel tail (lab=0 => masked).
Host: sums per-core u/y lane totals into per-image hinge sums and does
the final ~500-flop scalar assembly (centers, pair loss, reg, totals).
"""

import os
import sys

import numpy as np

sys.path.insert(0, "/opt/trn_rl_repo")
os.environ.setdefault("MYCRO_LOCAL_CACHE", "1")

import ml_dtypes  # noqa: E402

BF16 = ml_dtypes.bfloat16

# problem constants (hardcoded per harness contract)
B, E, H, W = 4, 16, 1024, 1024
NIMG = H * W
NCORES = 8
NPIX = NIMG // 2             # pixels per core
KC = 9                       # classes 1..9 (0 = background, excluded)
DELTA_VAR = 0.5
DELTA_DST = 1.5
A_W, B_W, R_W = 1.0, 1.0, 0.001
BIG = 1024.0

# pass-1 layout
F1 = 1024
NCH1 = 4                     # RUN=4096 pixels/partition = NCH1*F1
RUN = NPIX // 128

# pass-2 layout: chunks of 7 j-groups x 512 cols = 3584 px
F2 = 512
NCHUNK = 147                 # 146 full + 1 zero-padded tail chunk
N2COLS = NCHUNK * F2         # 75264 cols in the kj-interleaved layout
NSG = 25                     # supergroups: 24 x 6 chunks + 1 x 3 chunks
NLANE = 63                   # (9k, 7j), k-major: l = 7k + j

_cache = {}


def _consts():
    """Host-side constant input arrays shared by all cores."""
    # kpat: [128, 9*F1]: kpat[p, k*F1 + f] = k+1
    kpat = np.zeros((128, KC * F1), dtype=np.float32)
    for k in range(KC):
        kpat[:, k * F1:(k + 1) * F1] = float(k + 1)
    # s23: [126, 63] rows 0-111 (j,e): 1 -> lane (k,j); rows 112-118 lab row
    # j: 2*BIG*kk; rows 119-125 labsq row j: -BIG
    s23 = np.zeros((126, NLANE), dtype=np.float32)
    for j in range(7):
        for k in range(KC):
            kk = k + 1
            for e in range(E):
                s23[16 * j + e, 7 * k + j] = 1.0
            s23[112 + j, 7 * k + j] = 2.0 * BIG * kk
            s23[119 + j, 7 * k + j] = -BIG
    # qsel2: [9, 128]: qsel2[k, 7k+j] = qsel2[k, 64+7k+j] = 1
    qsel2 = np.zeros((KC, 128), dtype=np.float32)
    for k in range(KC):
        for j in range(7):
            qsel2[k, 7 * k + j] = 1.0
            qsel2[k, 64 + 7 * k + j] = 1.0
    # bk2: [128, 1] = -BIG*kk^2 - dvar^2; -1e30 on pad lanes 63/127
    bk2 = np.full((128, 1), -1e30, dtype=np.float32)
    for k in range(KC):
        kk = k + 1
        for j in range(7):
            v = -BIG * kk * kk - DELTA_VAR * DELTA_VAR
            bk2[7 * k + j, 0] = v
            bk2[64 + 7 * k + j, 0] = v
    id9 = np.eye(KC, dtype=np.float32)
    return {
        "kpat": kpat.astype(BF16),
        "s23": s23.astype(BF16),
        "qsel2": qsel2,
        "bk2": bk2,
        "id9": id9,
    }


def build_module():
    """Build the SPMD Bass module (same program on all 8 cores)."""
    import concourse.mybir as mybir
    import concourse.tile as tile
    from concourse import bacc

    f32 = mybir.dt.float32
    bf16 = mybir.dt.bfloat16
    Alu = mybir.AluOpType
    Act = mybir.ActivationFunctionType

    nc = bacc.Bacc(
        "TRN2",
        target_bir_lowering=False,
        debug=False,
        num_devices=NCORES,
    )

    # I/O
    em1_d = nc.dram_tensor("em1", [E, NPIX], bf16, kind="ExternalInput").ap()
    em2_d = nc.dram_tensor("em2", [112, N2COLS], bf16,
                           kind="ExternalInput").ap()
    labf_d = nc.dram_tensor("labf", [NPIX], bf16, kind="ExternalInput").ap()
    lab2_d = nc.dram_tensor("lab2", [14, N2COLS], bf16,
                            kind="ExternalInput").ap()
    kpat_d = nc.dram_tensor("kpat", [128, KC * F1], bf16,
                            kind="ExternalInput").ap()
    s23_d = nc.dram_tensor("s23", [126, NLANE], bf16,
                           kind="ExternalInput").ap()
    qsel2_d = nc.dram_tensor("qsel2", [KC, 128], f32,
                             kind="ExternalInput").ap()
    bk2_d = nc.dram_tensor("bk2", [128, 1], f32, kind="ExternalInput").ap()
    id9_d = nc.dram_tensor("id9", [KC, KC], f32, kind="ExternalInput").ap()

    stats_ext = nc.dram_tensor("stats", [KC, 17], f32,
                               kind="ExternalOutput").ap()
    uacc_d = nc.dram_tensor("uacc", [128, NSG], f32,
                            kind="ExternalOutput").ap()
    yacc_d = nc.dram_tensor("yacc", [128, NSG], f32,
                            kind="ExternalOutput").ap()

    with tile.TileContext(nc) as tc:
        with (
            tc.tile_pool(name="consts", bufs=1) as cp,
            tc.tile_pool(name="oh", bufs=2) as ohp_pool,
            tc.tile_pool(name="p2", bufs=4) as p2,
            tc.tile_pool(name="p2b", bufs=2) as p2b,
            tc.tile_pool(name="dram", bufs=1, space="DRAM") as dp,
        ):
            # ---- persistent constants ----
            kpat_t = cp.tile([128, KC * F1], bf16)
            kpat_dv = kpat_d.rearrange("p (k f) -> p k f", f=F1)
            kpat_tv = kpat_t[:].rearrange("p (k f) -> p k f", f=F1)
            nc.scalar.dma_start(kpat_tv[:, :, 0:512], kpat_dv[:, :, 0:512])
            nc.scalar.dma_start(kpat_tv[:, :, 512:F1], kpat_dv[:, :, 512:F1])
            s23_t = cp.tile([126, NLANE], bf16)
            nc.gpsimd.dma_start(s23_t[:], s23_d[:])
            qsel2_t = cp.tile([KC, 128], f32)
            nc.gpsimd.dma_start(qsel2_t[:], qsel2_d[:])
            bk2_t = cp.tile([128, 1], f32)
            nc.gpsimd.dma_start(bk2_t[:], bk2_d[:])
            id9_t = cp.tile([KC, KC], f32)
            nc.gpsimd.dma_start(id9_t[:], id9_d[:])
            dv2 = cp.tile([128, 1], f32)
            nc.vector.memset(dv2[:], DELTA_VAR * DELTA_VAR)

            # ---- labels pixel-major ----
            lab_pm = cp.tile([128, RUN], bf16)
            labf_r = labf_d.rearrange("(p c) -> p c", p=128)
            nc.sync.dma_start(lab_pm[:, 0:512], labf_r[:, 0:512])
            nc.sync.dma_start(lab_pm[:, 512:RUN], labf_r[:, 512:RUN])

            # ---- pass 1: one-hot segment sums + counts ----
            # embp: [128, 17*F1], slots 0-15 = emb (e-major), slot 16 = ones
            embps = []
            for i in range(2):
                t = cp.tile([128, 17 * F1], bf16, tag=f"embp{i}")
                nc.vector.memset(t[:, 16 * F1:17 * F1], 1.0)
                embps.append(t)

            ps1_cm = tc.tile_pool(name="ps1", bufs=1, space="PSUM")
            ps1 = ps1_cm.__enter__()
            psum1 = ps1.tile([KC, 17], f32)
            em1_r = em1_d.rearrange("e (p c f) -> c p e f", p=128, c=NCH1,
                                    f=F1)
            kpat_v = kpat_t[:].rearrange("p (k f) -> p k f", f=F1)
            HF = F1 // 2
            for c in range(NCH1):
                embp = embps[c % 2]
                embp_s = embp[:].rearrange("p (s f) -> p s f", f=F1)
                ohp = ohp_pool.tile([128, KC * F1], bf16, tag="ohp")
                ohp_s = ohp[:].rearrange("p (k f) -> p k f", f=F1)
                # split loads/one-hot by column half so MMs start earlier
                for h in range(2):
                    fs = slice(h * HF, (h + 1) * HF)
                    nc.sync.dma_start(embp_s[:, 0:16, fs], em1_r[c][:, :, fs])
                    nc.vector.tensor_tensor(
                        out=ohp_s[:, :, fs],
                        in0=lab_pm[:, c * F1 + h * HF:c * F1 + (h + 1) * HF]
                            .unsqueeze(1).to_broadcast([128, KC, HF]),
                        in1=kpat_v[:, :, fs],
                        op=Alu.is_equal,
                    )
                ohp_v = ohp[:].rearrange("p (k f) -> p f k", f=F1)
                embp_v = embp[:].rearrange("p (s f) -> p f s", f=F1)
                for f in range(F1):
                    nc.tensor.matmul(
                        psum1[:],
                        lhsT=ohp_v[:, f, :],
                        rhs=embp_v[:, f, :],
                        start=(c == 0 and f == 0),
                        stop=(c == NCH1 - 1 and f == F1 - 1),
                    )

            stats_sb = cp.tile([KC, 17], f32)
            nc.scalar.copy(stats_sb[:], psum1[:])

            # ---- AllReduce stats over the 2-core image pair ----
            cc_in = dp.tile([KC, 17], f32)
            cc_out = dp.tile([KC, 17], f32)
            nc.scalar.dma_start(cc_in[:], stats_sb[:])
            nc.gpsimd.collective_compute(
                "AllReduce",
                mybir.AluOpType.add,
                replica_groups=[[0, 1], [2, 3], [4, 5], [6, 7]],
                ins=[cc_in[:].opt()],
                outs=[cc_out[:].opt()],
            )
            myst = cp.tile([KC, 17], f32)
            nc.scalar.dma_start(myst[:], cc_out[:])
            nc.scalar.dma_start(stats_ext[:], cc_out[:])

            # ---- centers, q, stationaries for pass 2 ----
            cnt_safe = cp.tile([KC, 1], f32)
            nc.vector.tensor_scalar(out=cnt_safe[:], in0=myst[:, 16:17],
                                    scalar1=1.0, scalar2=None, op0=Alu.max)
            rec = cp.tile([KC, 1], f32)
            nc.vector.reciprocal(rec[:], cnt_safe[:])
            cmat = cp.tile([KC, E], f32)
            nc.vector.tensor_scalar(out=cmat[:], in0=myst[:, 0:16],
                                    scalar1=rec[:, 0:1], scalar2=None,
                                    op0=Alu.mult)
            csq = cp.tile([KC, E], f32)
            nc.vector.tensor_tensor(csq[:], cmat[:], cmat[:], op=Alu.mult)
            qv = cp.tile([KC, 1], f32)
            nc.vector.tensor_reduce(qv[:], csq[:], mybir.AxisListType.X,
                                    Alu.add)

            ct_ps = ps1.tile([E, KC], f32)
            nc.tensor.matmul(ct_ps[:], lhsT=cmat[:], rhs=id9_t[:],
                             start=True, stop=True)
            ctbm = cp.tile([E, KC], bf16)
            nc.vector.tensor_scalar(out=ctbm[:], in0=ct_ps[:], scalar1=-2.0,
                                    scalar2=None, op0=Alu.mult)
            # s1: [112, 63]: s1[16j+e, 7k+j] = -2*C[kk,e]
            s1_t = cp.tile([112, NLANE], bf16)
            nc.vector.memset(s1_t[:], 0.0)
            s1_v = s1_t[:].rearrange("p (k j) -> p j k", j=7)
            for j in range(7):
                nc.scalar.dma_start(s1_v[16 * j:16 * (j + 1), j, :], ctbm[:])

            qb_ps = ps1.tile([128, 1], f32)
            nc.tensor.matmul(qb_ps[:], lhsT=qsel2_t[:], rhs=qv[:],
                             start=True, stop=True)
            qb2 = cp.tile([128, 1], f32)
            nc.vector.tensor_tensor(qb2[:], qb_ps[:], bk2_t[:], op=Alu.add)
            ps1_cm.__exit__(None, None, None)

            # ---- pass 2 ----
            psp_cm = tc.tile_pool(name="ps2", bufs=2, space="PSUM")
            psp = psp_cm.__enter__()
            uacc = cp.tile([128, NSG], f32)
            yacc = cp.tile([128, NSG], f32)
            for g in range(NSG):
                nch = 6 if g < NSG - 1 else 3
                ncol = nch * F2
                col0 = g * 6 * F2
                embB = p2.tile([112, 6 * F2], bf16, tag="embB")
                nc.sync.dma_start(embB[:, 0:ncol],
                                  em2_d[:, col0:col0 + ncol])
                e2s = p2.tile([126, 6 * F2], bf16, tag="e2s")
                nc.gpsimd.dma_start(e2s[112:126, 0:ncol],
                                    lab2_d[:, col0:col0 + ncol])
                nc.vector.tensor_tensor(e2s[0:112, 0:ncol], embB[:, 0:ncol],
                                        embB[:, 0:ncol], op=Alu.mult)

                nlp = 128 if nch == 6 else NLANE
                ps2t = psp.tile([128, 3 * F2], f32, tag="ps2")
                for s in range(nch):
                    half, s3 = s // 3, s % 3
                    base = 64 * half
                    out = ps2t[base:base + NLANE, s3 * F2:(s3 + 1) * F2]
                    mv = slice(s * F2, (s + 1) * F2)
                    nc.tensor.matmul(out, lhsT=s1_t[:], rhs=embB[:, mv],
                                     start=True, stop=False,
                                     tile_position=(0, base))
                    nc.tensor.matmul(out, lhsT=s23_t[:], rhs=e2s[:, mv],
                                     start=False, stop=True,
                                     tile_position=(0, base))

                u_t = p2b.tile([128, 3 * F2], bf16, tag="u")
                nc.scalar.activation(u_t[0:nlp, :], ps2t[0:nlp, :], Act.Relu,
                                     bias=qb2[0:nlp, 0:1], scale=1.0,
                                     accum_out=uacc[0:nlp, g:g + 1])
                y_t = p2b.tile([128, 3 * F2], bf16, tag="y")
                nc.scalar.activation(y_t[0:nlp, :], u_t[0:nlp, :], Act.Sqrt,
                                     bias=dv2[0:nlp, 0:1], scale=1.0,
                                     accum_out=yacc[0:nlp, g:g + 1])

            nc.gpsimd.dma_start(uacc_d[:], uacc[:])
            nc.gpsimd.dma_start(yacc_d[:], yacc[:])
            psp_cm.__exit__(None, None, None)

    nc.compile()
    return nc


def _prep_core(esh32, lsh):
    """Build per-core input arrays from the f32 [E, NPIX] shard + labels."""
    e1 = esh32.astype(BF16)                              # [16, NPIX]
    labf = lsh.astype(BF16)                              # [NPIX]
    nfull = 146 * 7 * F2                                 # 523264
    v = e1[:, :nfull].reshape(E, 146, 7, F2).transpose(2, 0, 1, 3)
    em2 = np.zeros((112, N2COLS), dtype=BF16)
    em2[:, :146 * F2] = np.ascontiguousarray(v.reshape(112, 146 * F2))
    tail = e1[:, nfull:].reshape(E, 2, F2).transpose(1, 0, 2)  # [2, 16, 512]
    em2[0:32, 146 * F2:147 * F2] = tail.reshape(32, F2)
    lv = labf[:nfull].reshape(146, 7, F2).transpose(1, 0, 2)
    lab2 = np.zeros((14, N2COLS), dtype=BF16)
    lab2[0:7, :146 * F2] = np.ascontiguousarray(lv.reshape(7, 146 * F2))
    lab2[0:2, 146 * F2:147 * F2] = labf[nfull:].reshape(2, F2)
    lab2[7:14, :] = lab2[0:7, :] * lab2[0:7, :]
    return {"em1": np.ascontiguousarray(e1), "em2": em2,
            "labf": labf, "lab2": lab2}


def _host_finalize(stats, hsum):
    """stats: [B, 9, 17] (per-image); hsum: [B, 9] hinge sums (classes 1..9)."""
    lv_l, ld_l, lr_l, valid_l = [], [], [], []
    for b in range(B):
        counts = stats[b, :, 16].astype(np.float64)
        sums = stats[b, :, 0:16].astype(np.float64)
        present = counts > 0                      # classes 1..9
        presf = present.astype(np.float64)
        safe = np.where(counts > 0, counts, 1.0)
        centers = sums / safe[:, None]
        per_inst = hsum[b].astype(np.float64) / safe
        n_inst = presf.sum()
        lv = float((per_inst * presf).sum() / max(n_inst, 1.0))
        cdiff = centers[:, None, :] - centers[None, :, :]
        csq = (cdiff * cdiff).sum(-1)
        ids = np.arange(1, KC + 1)
        pm = present[:, None] & present[None, :] & (ids[:, None] < ids[None, :])
        cdist = np.sqrt(np.where(pm, csq, 1.0))
        ph = np.square(np.maximum(2.0 * DELTA_DST - cdist, 0.0)) * pm
        n_pairs = pm.sum()
        ld = float(ph.sum() / max(n_pairs, 1.0))
        cn = np.sqrt(np.where(present, (centers * centers).sum(-1), 1.0))
        lr = float((cn * presf).sum() / max(n_inst, 1.0))
        valid = 1.0 if n_inst > 0 else 0.0
        lv_l.append(lv * valid)
        ld_l.append(ld * valid)
        lr_l.append(lr * valid)
        valid_l.append(valid)
    vb = max(sum(valid_l), 1.0)
    loss_var = sum(lv_l) / vb
    loss_dst = sum(ld_l) / vb
    loss_reg = sum(lr_l) / vb
    total = A_W * loss_var + B_W * loss_dst + R_W * loss_reg
    return (
        np.float32(total),
        np.float32(loss_var),
        np.float32(loss_dst),
        np.float32(loss_reg),
    )


def _core_hinge(uacc, yacc):
    """Per-core H[k] for classes 1..9 from the [128, NSG] accum tiles."""
    u = uacc.astype(np.float64)
    y = yacc.astype(np.float64)
    hk = np.zeros(KC)
    npp = float(NCHUNK * F2)          # pixels streamed per (k,j) lane pair
    for k in range(KC):
        for j in range(7):
            lo, hi = 7 * k + j, 64 + 7 * k + j
            ut = u[lo, :].sum() + u[hi, 0:NSG - 1].sum()
            yt = y[lo, :].sum() + y[hi, 0:NSG - 1].sum()
            hk[k] += (ut - 2.0 * DELTA_VAR * yt
                      + 2.0 * DELTA_VAR * DELTA_VAR * npp)
    return hk


def kernel(embedding, ins_label):
    from concourse.bass_utils import run_bass_kernel_spmd

    key = "mod"
    if key not in _cache:
        _cache[key] = build_module()
    nc = _cache[key]

    consts = _consts()
    emb_r = np.asarray(embedding, dtype=np.float32).reshape(B, E, NIMG)
    lab_r = np.asarray(ins_label).reshape(B, NIMG).astype(np.float32)

    in_maps = []
    for c in range(NCORES):
        b, h = c // 2, c % 2
        sl = slice(h * NPIX, (h + 1) * NPIX)
        m = dict(consts)
        m.update(_prep_core(emb_r[b, :, sl], lab_r[b, sl]))
        in_maps.append(m)

    res = run_bass_kernel_spmd(nc, in_maps, core_ids=list(range(NCORES)))
    global LAST_RESULTS
    LAST_RESULTS = res

    stats = np.stack([res.results[2 * b]["stats"].astype(np.float64)
                      for b in range(B)])
    hsum = np.zeros((B, KC))
    for c in range(NCORES):
        hsum[c // 2] += _core_hinge(res.results[c]["uacc"],
                                    res.results[c]["yacc"])
    return _host_finalize(stats.astype(np.float64), hsum)


if __name__ == "__main__":
    build_module()
    print("build ok")
